# revision 15
# baseline (speedup 1.0000x reference)
"""Trainium2 Bass kernel for nn_ODEFunc_88321707475429 (gnn_message_passing).

Data-parallel over batch B=32 across 8 NeuronCores (B_local=4 per core).
Per-core pipeline (all fp16 data, fp32 PSUM accumulation):
  x0 row-major [N, 256] (col = b*64 + c, b-major) in a DRAM arena of 4 slots.
  SpMM = dma_gather of neighbor rows (512B descriptors) + per-128-edge-chunk
  PE matmuls against a host-built one-hot [128, 32] (edge weight at the
  window-local output-row column), accumulated in PSUM per 32-row window.
  Chebyshev second stage (2*S*x1 - x0) folds the 2x into the one-hot values
  and the -x0 via extra self-edges gathering slot 0.
  Stage-1 output: row-major (re-gathered by stage 2) + PE-transposed copy.
  Stage-2 output: produced directly channel-major (G^T @ onehot).
  Channel-major xs^T feed k=64 PE matmuls with W slices; fused bias+
  sigmoid/tanh on the scalar engine; final -theta*c transposed back
  row-major via PE transpose with a negated identity.
"""

import sys

sys.path.insert(0, "/opt/trn_rl_repo")

import numpy as np

N = 8192
DEG = 32
L = 64
U = 64
M = 5
B = 32
BL = 4          # batches per core
C = BL * L      # 256 row width
W32 = 32        # rows per output window
NWIN = N // W32
SLAB = 1024     # gather indices per dma_gather (HW single-packet idx limit)
NT = N // 128   # 64 node tiles


# ---------------------------------------------------------------- tile patch
def _patch_tile_drain():
    """This walrus build rejects instructions with >1 sync wait; hang the
    TileContext tail-drain waits on single-wait nop carriers instead."""
    import concourse.tile as tile
    from concourse.vector_clock import ScopedClock, VectorClock

    def _drain_and_barrier(self, tick_clock, wait_clock):
        gc = tick_clock.global_clock
        vec = list(gc)
        n = len(vec)
        for i in range(n):
            if not vec[i]:
                continue
            sub = [0] * n
            sub[i] = vec[i]
            nop_inst = self.nc.sync.nop(nofuse=True, hint="tail_drain_wait")
            wait_clock.add_sem_waits(
                nop_inst.ins, ScopedClock({None: VectorClock(sub)})
            )
        self.nc.sync.drain()
        self.nc.all_engine_barrier()
        assert self.sems is not None
        popped = self.nc._tile_sem_poison_stack.pop()
        assert popped is self._sem_poison
        self.nc.clear_and_free_semaphores(list(self.sems.allocated().values()))
        self.nc.all_engine_barrier()

    tile.TileContext._drain_and_barrier = _drain_and_barrier


# ------------------------------------------------------------- host edge prep
def _prep_support(rows, cols, vals, stage, slot):
    """Sorted-by-row padded edge list for one (support, stage).

    stage 2: vals*2, gather from slot `slot`, plus self-edges (r, r@slot0, -1).
    Window (32 rows) edge counts padded to multiples of 128 so every 128-edge
    chunk maps to exactly one window."""
    rows = np.asarray(rows, np.int64)
    cols = np.asarray(cols, np.int64)
    vals = np.asarray(vals, np.float64)
    if stage == 2:
        rows = np.concatenate([rows, np.arange(N)])
        cols = np.concatenate([cols + slot * N, np.arange(N)])
        vals = np.concatenate([2.0 * vals, -np.ones(N)])
    o = np.argsort(rows, kind="stable")
    r, c, v = rows[o], cols[o], vals[o]
    counts = np.bincount(r, minlength=N)
    wcounts = counts.reshape(NWIN, W32).sum(1)
    pcounts = ((wcounts + 127) // 128) * 128
    total = int(pcounts.sum())
    idx = np.zeros(total, np.int16)
    val = np.zeros(total, np.float32)
    lrow = np.zeros(total, np.int32)
    wstart_e = np.concatenate([[0], np.cumsum(wcounts)])
    wstart_p = np.concatenate([[0], np.cumsum(pcounts)])
    for w in range(NWIN):
        e0, e1 = wstart_e[w], wstart_e[w + 1]
        p0 = wstart_p[w]
        n = e1 - e0
        idx[p0 : p0 + n] = c[e0:e1].astype(np.int16)
        val[p0 : p0 + n] = v[e0:e1].astype(np.float32)
        lrow[p0 : p0 + n] = (r[e0:e1] - w * W32).astype(np.int32)
    nchunk = total // 128
    oh = np.zeros((128, nchunk, W32), np.float16)
    e = np.arange(total)
    oh[e % 128, e // 128, lrow] = val.astype(np.float16)
    idx_w = np.tile(idx.reshape(-1, 16).T, (8, 1)).astype(np.int16)
    cpw = (pcounts // 128).astype(np.int64).tolist()
    return dict(idx_w=idx_w, oh=np.ascontiguousarray(oh.reshape(128, -1)),
                cpw=cpw, nchunk=nchunk)


def _legalize_waits(nc):
    """This walrus build accepts at most one sync wait per instruction.
    Move extra waits onto single-wait same-engine NoOp carriers inserted
    immediately before the owning instruction."""
    import concourse.mybir as mybir

    cnt = 0
    for f in nc.m.functions:
        for bb in f.blocks:
            out = []
            for ins in bb.instructions:
                si = ins.sync_info
                if si is not None and len(si.on_wait) > 1:
                    waits = list(si.on_wait)
                    for w in waits[:-1]:
                        cnt += 1
                        nop = mybir.InstNoOp(
                            name=f"waitnop_{cnt}", ins=[], outs=[]
                        )
                        nop.engine = ins.engine
                        nop.sync_info = mybir.SyncInfo(on_wait=[w],
                                                       on_update=[])
                        out.append(nop)
                    ins.sync_info = mybir.SyncInfo(
                        on_wait=[waits[-1]], on_update=list(si.on_update)
                    )
                out.append(ins)
            bb.instructions = out
    return cnt


# ------------------------------------------------------------- program build
def _build_program(eds, legalize=True):
    import concourse.bass as bass
    import concourse.mybir as mybir
    import concourse.tile as tile
    from concourse import library_config
    from concourse.masks import make_identity

    f16 = mybir.dt.float16
    f32 = mybir.dt.float32
    i16 = mybir.dt.int16

    nc = bass.Bass()
    x0row_d = nc.dram_tensor("x0row", [N, C], f16, kind="ExternalInput")
    arena_d = nc.dram_tensor("arena", [4 * N, C], f16)
    out_d = nc.dram_tensor("out", [N, C], f32, kind="ExternalOutput")
    bt_d = nc.dram_tensor("bt", [L, 1], f32, kind="ExternalInput")
    b0_d = nc.dram_tensor("b0", [L, 1], f32, kind="ExternalInput")

    idx_d, oh_d = {}, {}
    for key, ed in eds.items():
        s, st = key
        T = ed["nchunk"] * 128
        idx_d[key] = nc.dram_tensor(f"idx_{s}_{st}", [128, T // 16], i16,
                                    kind="ExternalInput")
        oh_d[key] = nc.dram_tensor(f"oh_{s}_{st}", [128, ed["nchunk"] * W32],
                                   f16, kind="ExternalInput")
    # replicated weight slices [128, 64]: two stacked copies of W_m
    w_d = {}
    for o in ("t", "0", "f"):
        for m5 in range(M):
            w_d[(o, m5)] = nc.dram_tensor(f"w{o}_{m5}", [128, L], f16,
                                          kind="ExternalInput")
    xsT_d = [nc.dram_tensor(f"xsT{m5}", [C, N], f16) for m5 in range(M)]
    theta_d = nc.dram_tensor("theta", [L, BL * N], f16)

    with tile.TileContext(nc) as tc:
        nc.gpsimd.load_library(library_config.mlp)
        with (
            tc.tile_pool(name="sb", bufs=3) as pool,
            tc.tile_pool(name="slab", bufs=3) as spool,
            tc.tile_pool(name="persist", bufs=1) as ppool,
            tc.tile_pool(name="xpool", bufs=1) as xpool,
            tc.tile_pool(name="ps", bufs=2, space="PSUM") as pspool,
        ):
            ident = ppool.tile([128, 128], f16, tag="ident")
            make_identity(nc, ident[:])
            ident32 = ppool.tile([128, 128], f32, tag="ident32")
            make_identity(nc, ident32[:])
            bt_t = ppool.tile([L, 1], f32, tag="bt")
            nc.sync.dma_start(out=bt_t[:], in_=bt_d[:])
            b0_t = ppool.tile([L, 1], f32, tag="b0")
            nc.sync.dma_start(out=b0_t[:], in_=b0_d[:])
            w_t = {}
            for key, d in w_d.items():
                w_t[key] = ppool.tile([128, L], f16, tag=f"w{key[0]}{key[1]}",
                                      name=f"w{key[0]}{key[1]}")
                nc.sync.dma_start(out=w_t[key][:], in_=d[:])

            def chain(inst):
                tc.chain_iter_dep("arena", inst.ins)

            reg_cache = {}

            def nidx_reg(v):
                if v not in reg_cache:
                    reg_cache[v] = nc.gpsimd.to_reg(v)
                return reg_cache[v]

            # ---- gather slab machinery -------------------------------------
            def slab_iter(key):
                """Yield (g_tile, oh_tile, slot) per chunk, DMAing slabs of
                SLAB indices as needed."""
                ed = eds[key]
                nchunk = ed["nchunk"]
                cur = [None, None]
                for k in range(nchunk):
                    slot = k % (SLAB // 128)
                    if slot == 0:
                        nleft = min(SLAB // 128, nchunk - k)
                        nidx = nleft * 128
                        idx_t = spool.tile([128, SLAB // 16], i16, tag="idx")
                        nc.sync.dma_start(
                            out=idx_t[:, : nidx // 16],
                            in_=idx_d[key][:, k * 8 : k * 8 + nidx // 16],
                        )
                        g_t = spool.tile([128, SLAB // 128 * C], f16, tag="g")
                        gi = nc.gpsimd.dma_gather(
                            out_ap=g_t[:, : nleft * C].rearrange(
                                "p (k e) -> p k e", e=C
                            ),
                            in_ap=arena_d[:],
                            idxs_ap=idx_t[:, : nidx // 16],
                            num_idxs=nidx,
                            num_idxs_reg=nidx_reg(nidx),
                            elem_size=C,
                        )
                        chain(gi)
                        oh_t = spool.tile([128, SLAB // 128 * W32], f16,
                                          tag="oh")
                        nc.sync.dma_start(
                            out=oh_t[:, : nleft * W32],
                            in_=oh_d[key][:, k * W32 : (k + nleft) * W32],
                        )
                        cur = [g_t, oh_t]
                    yield cur[0], cur[1], slot

            # ---- spmm stage 1: row-major out + transpose to xsT ------------
            def spmm_stage1(key, out_slot, xsT):
                ed = eds[key]
                cpw = ed["cpw"]
                it = slab_iter(key)
                k = 0
                for blk in range(NT):       # 128-row PSUM bank groups
                    ps = pspool.tile([128, C], f32, tag="bank", space="PSUM")
                    for wi in range(4):     # windows in bank
                        w = blk * 4 + wi
                        nck = cpw[w]
                        for j in range(nck):
                            g_t, oh_t, slot = next(it)
                            nc.tensor.matmul(
                                out=ps[W32 * wi : W32 * (wi + 1), :],
                                lhsT=oh_t[:, slot * W32 : (slot + 1) * W32],
                                rhs=g_t[:, slot * C : (slot + 1) * C],
                                start=(j == 0),
                                stop=(j == nck - 1),
                                tile_position=(0, W32 * wi),
                            )
                            k += 1
                    sb = pool.tile([128, C], f16, tag="st1row")
                    nc.vector.tensor_copy(out=sb[:], in_=ps[:])
                    wr = nc.sync.dma_start(
                        out=arena_d[out_slot * N + blk * 128 :
                                    out_slot * N + (blk + 1) * 128, :],
                        in_=sb[:],
                    )
                    chain(wr)
                    for h in range(2):
                        pst = pspool.tile([128, 128], f16, tag="tr",
                                          space="PSUM")
                        nc.tensor.transpose(
                            out=pst[:],
                            in_=sb[:, 128 * h : 128 * (h + 1)],
                            identity=ident[:],
                        )
                        sbt = pool.tile([128, 128], f16, tag="st1t")
                        nc.vector.tensor_copy(out=sbt[:], in_=pst[:])
                        nc.sync.dma_start(
                            out=xsT[128 * h : 128 * (h + 1),
                                    blk * 128 : (blk + 1) * 128],
                            in_=sbt[:],
                        )

            # ---- spmm stage 2: channel-major out (G^T @ onehot) ------------
            def spmm_stage2(key, xsT):
                ed = eds[key]
                cpw = ed["cpw"]
                it = slab_iter(key)
                for grp in range(NWIN // 16):   # 16 windows = 512 rows
                    psT = [pspool.tile([128, 512], f32, tag=f"bankT{h}",
                                       name=f"psT{h}", space="PSUM") for h in range(2)]
                    for wi in range(16):
                        w = grp * 16 + wi
                        nck = cpw[w]
                        for j in range(nck):
                            g_t, oh_t, slot = next(it)
                            for h in range(2):
                                nc.tensor.matmul(
                                    out=psT[h][:, W32 * wi : W32 * (wi + 1)],
                                    lhsT=g_t[:, slot * C + 128 * h :
                                             slot * C + 128 * (h + 1)],
                                    rhs=oh_t[:, slot * W32 : (slot + 1) * W32],
                                    start=(j == 0),
                                    stop=(j == nck - 1),
                                    tile_position=(0, 0),
                                )
                    for h in range(2):
                        sb = pool.tile([128, 512], f16, tag="st2t")
                        nc.vector.tensor_copy(out=sb[:], in_=psT[h][:])
                        nc.sync.dma_start(
                            out=xsT[128 * h : 128 * (h + 1),
                                    grp * 512 : (grp + 1) * 512],
                            in_=sb[:],
                        )

            # ---- x0 -> xsT[0] via DMA transpose ----------------------------
            def load_x0T(src_d, from_arena=False):
                for h in range(2):
                    sbt = xpool.tile([128, N], f16, tag="x0T")
                    rd = nc.sync.dma_start(
                        out=sbt[:],
                        in_=src_d[0:N, 128 * h : 128 * (h + 1)],
                        transpose=True,
                    )
                    if from_arena:
                        chain(rd)
                    nc.sync.dma_start(
                        out=xsT_d[0][128 * h : 128 * (h + 1), :], in_=sbt[:]
                    )

            # ---- final gconv matmuls + activations -------------------------
            mybir_ = mybir

            def final_phase(is_c_set):
                AF = mybir_.ActivationFunctionType
                outs = [("f", AF.Tanh, bt_t)] if is_c_set else [
                    ("t", AF.Sigmoid, bt_t), ("0", AF.Tanh, b0_t)]
                for t in range(NT):
                    xt = {}
                    for m5 in range(M):
                        for h in range(2):
                            tl = pool.tile([128, 128], f16, tag=f"xt{m5}{h}")
                            nc.sync.dma_start(
                                out=tl[:],
                                in_=xsT_d[m5][128 * h : 128 * (h + 1),
                                              t * 128 : (t + 1) * 128],
                            )
                            xt[(m5, h)] = tl
                    rowtile = None
                    if not is_c_set:
                        rowtile = pool.tile([128, C], f16, tag="c0row")
                    outtile = None
                    if is_c_set:
                        outtile = pool.tile([128, C], f32, tag="outrow")
                    for b in range(BL):
                        h, q = b // 2, b % 2
                        for o, af, bias in outs:
                            po = pspool.tile([64, 128], f32, tag="tr",
                                             space="PSUM")
                            for m5 in range(M):
                                nc.tensor.matmul(
                                    out=po[:],
                                    lhsT=w_t[(o, m5)][64 * q : 64 * (q + 1), :],
                                    rhs=xt[(m5, h)][64 * q : 64 * (q + 1), :],
                                    start=(m5 == 0),
                                    stop=(m5 == M - 1),
                                    tile_position=(64 * q, 0),
                                )
                            if o == "t":
                                tha = pool.tile([L, 128], f16, tag="thact")
                                nc.scalar.activation(out=tha[:], in_=po[:],
                                                     func=af, bias=bias[:])
                                nc.sync.dma_start(
                                    out=theta_d[:, b * N + t * 128 :
                                                b * N + (t + 1) * 128],
                                    in_=tha[:],
                                )
                            elif o == "0":
                                act = pool.tile([64, 128], f16, tag="c0act")
                                nc.scalar.activation(out=act[:], in_=po[:],
                                                     func=af, bias=bias[:])
                                ptr = pspool.tile([128, 64], f16, tag="bank",
                                                  space="PSUM")
                                nc.tensor.transpose(
                                    out=ptr[:], in_=act[:],
                                    identity=ident[0:64, 0:64],
                                )
                                nc.vector.tensor_copy(
                                    out=rowtile[:, 64 * b : 64 * (b + 1)],
                                    in_=ptr[:],
                                )
                            else:  # final output: -theta * tanh(...)
                                act = pool.tile([64, 128], f32, tag="cfact")
                                nc.scalar.activation(out=act[:], in_=po[:],
                                                     func=af, bias=bias[:])
                                thr = pool.tile([L, 128], f16, tag="thrd")
                                nc.sync.dma_start(
                                    out=thr[:],
                                    in_=theta_d[:, b * N + t * 128 :
                                                b * N + (t + 1) * 128],
                                )
                                prod = pool.tile([64, 128], f32, tag="prod")
                                nc.vector.tensor_mul(
                                    out=prod[:], in0=act[:], in1=thr[:],
                                )
                                ptr = pspool.tile([128, 64], f32, tag="bank",
                                                  space="PSUM")
                                nc.tensor.transpose(
                                    out=ptr[:], in_=prod[:],
                                    identity=ident32[0:64, 0:64],
                                )
                                nc.vector.tensor_scalar_mul(
                                    out=outtile[:, 64 * b : 64 * (b + 1)],
                                    in0=ptr[:],
                                    scalar1=-1.0,
                                )
                    if not is_c_set:
                        wr = nc.sync.dma_start(
                            out=arena_d[t * 128 : (t + 1) * 128, :],
                            in_=rowtile[:],
                        )
                        chain(wr)
                    else:
                        nc.sync.dma_start(
                            out=out_d[t * 128 : (t + 1) * 128, :],
                            in_=outtile[:],
                        )

            # ---------------- schedule ----------------
            cp = nc.sync.dma_start(out=arena_d[0:N, :], in_=x0row_d[:])
            chain(cp)
            load_x0T(x0row_d)
            spmm_stage1((1, 1), 1, xsT_d[1])
            spmm_stage2((1, 2), xsT_d[2])
            spmm_stage1((2, 1), 2, xsT_d[3])
            spmm_stage2((2, 2), xsT_d[4])
            final_phase(False)
            # c-set: arena slot0 now holds c0 (written by final_phase)
            load_x0T(arena_d, from_arena=True)
            spmm_stage1((1, 1), 1, xsT_d[1])
            spmm_stage2((1, 2), xsT_d[2])
            spmm_stage1((2, 1), 2, xsT_d[3])
            spmm_stage2((2, 2), xsT_d[4])
            final_phase(True)

    if legalize:
        _legalize_waits(nc)
    import concourse.mybir as mybir2
    mybir2.codegen_inst_isa_subclasses(nc)
    return nc


_CACHE = {}


def _get_program_and_inputs(inputs):
    key = "prog"
    if key in _CACHE:
        return _CACHE[key]
    _patch_tile_drain()
    eds = {
        (1, 1): _prep_support(inputs["rows1"], inputs["cols1"],
                              inputs["vals1"], 1, 1),
        (1, 2): _prep_support(inputs["rows1"], inputs["cols1"],
                              inputs["vals1"], 2, 1),
        (2, 1): _prep_support(inputs["rows2"], inputs["cols2"],
                              inputs["vals2"], 1, 2),
        (2, 2): _prep_support(inputs["rows2"], inputs["cols2"],
                              inputs["vals2"], 2, 2),
    }
    nc = _build_program(eds)
    _CACHE[key] = (nc, eds)
    return nc, eds


def kernel(y, rows1, cols1, vals1, rows2, cols2, vals2, Wt, bt, W0, b0, Wf):
    inputs = dict(y=np.asarray(y, np.float32), rows1=np.asarray(rows1),
                  cols1=np.asarray(cols1), vals1=np.asarray(vals1),
                  rows2=np.asarray(rows2), cols2=np.asarray(cols2),
                  vals2=np.asarray(vals2))
    nc, eds = _get_program_and_inputs(inputs)

    from concourse.bass_utils import run_bass_kernel_spmd

    Wt, W0, Wf = (np.asarray(w, np.float32) for w in (Wt, W0, Wf))
    bt_in = np.asarray(bt, np.float32).reshape(L, 1)
    b0_in = np.asarray(b0, np.float32).reshape(L, 1)

    def wrep(Wfull, m5):
        wm = Wfull[m5::M].astype(np.float16)      # [64, 64]
        return np.concatenate([wm, wm], axis=0)   # [128, 64]

    common = dict(bt=bt_in, b0=b0_in)
    for key, ed in eds.items():
        s, st = key
        common[f"idx_{s}_{st}"] = ed["idx_w"]
        common[f"oh_{s}_{st}"] = ed["oh"]
    for o, Wfull in (("t", Wt), ("0", W0), ("f", Wf)):
        for m5 in range(M):
            common[f"w{o}_{m5}"] = wrep(Wfull, m5)

    in_maps = []
    yv = inputs["y"].reshape(B, N, L)
    for core in range(8):
        yb = yv[core * BL : (core + 1) * BL]
        x0 = np.ascontiguousarray(yb.transpose(1, 0, 2)).reshape(N, C)
        in_maps.append(dict(common, x0row=x0.astype(np.float16)))

    import time as _time

    _t0 = _time.time()
    res = run_bass_kernel_spmd(nc, in_maps, core_ids=list(range(8)))
    globals()["LAST_EXEC_NS"] = int((_time.time() - _t0) * 1e9)

    out = np.empty((B, N * L), np.float32)
    for core in range(8):
        o = res.results[core]["out"]          # [N, C] = [n, (b, u)]
        o = o.reshape(N, BL, L).transpose(1, 0, 2).reshape(BL, N * L)
        out[core * BL : (core + 1) * BL] = o
    return out
